# revision 22
# baseline (speedup 1.0000x reference)
"""Trainium2 Bass kernel for the 12-layer residual MLP (nn_Net_40321152975542).

Network (per row of x [B=2097152, 4]):
    h = relu(x @ W1.T + b1)                       # fc1: 4 -> 16
    res = h
    5x: h = relu(h @ Wa.T + ba)                   # A-layer 16 -> 16
        h = relu(h @ Wb.T + bb + res); res = h    # B-layer 16 -> 16 (+residual)
    y = h @ Wo.T + bo                             # head: 16 -> 2

Mapping: pure data-parallel across 8 NeuronCores (batch split).  On each core,
8 batch groups x 16 features are packed across the 128 SBUF partitions and 512
batch elements along the free dim (one "macro tile" = 4096 rows).  Every layer
is one 128x128 block-diagonal bf16 matmul; residual adds are identity matmuls
accumulated into the same PSUM bank; relu+bias runs as a single PSUM->SBUF pass
alternating between ScalarE (activation) and VectorE (tensor_scalar add+max).
The head writes PSUM->DRAM directly; bo is added on the host.

Host-side prep is layout only (transpose/cast/shard) so all device DMAs are
contiguous per partition.
"""

import os
import sys

sys.path.insert(0, "/opt/trn_rl_repo")

import numpy as np
import ml_dtypes
from contextlib import ExitStack

from concourse import bass, bacc, tile, mybir
from concourse.bass_utils import run_bass_kernel_spmd

BF16 = ml_dtypes.bfloat16

B = 2097152
N_CORES = 8
R = B // N_CORES          # rows per core
N = 512                   # free-dim columns per macro tile
G = 8                     # batch groups packed along partitions
H = 16                    # hidden width
MACRO = G * N             # rows per macro tile (4096)
N_MACROS = R // MACRO     # 64

# which relu passes run on ScalarE (rest on VectorE); 11 passes total
ACT_PASSES = frozenset({0, 2, 4, 6, 8, 10})

# best known full-size configuration (see dev_sweep / TimelineSim + HW slope)
BEST = dict(
    ncols=512,
    n_macros=R // (G * 512),
    skew=2,
    p_bufs=6,
    hd_bufs=2,
    h_bufs=14,
    x_bufs=8,
    y_bufs=3,
    head_merge=1,
    xlead=8,
)

_DT_BF16 = mybir.dt.bfloat16
_DT_F32 = mybir.dt.float32


def _prep_weights(W1, b1, Wh, bh, Wo, bo):
    """Build block-diagonal bf16 stationaries + replicated fp32 bias vectors."""
    w1s = np.zeros((32, 128), dtype=BF16)
    for g in range(G):
        w1s[4 * g : 4 * g + 4, H * g : H * g + H] = W1.T.astype(BF16)

    # K-major concat [128, 10*128]: one DMA, layer l stationary = [:, 128l:128l+128]
    wab = np.zeros((128, 10 * 128), dtype=BF16)
    for l in range(10):
        for g in range(G):
            wab[H * g : H * g + H, 128 * l + H * g : 128 * l + H * g + H] = (
                Wh[l].T.astype(BF16)
            )

    ids = np.eye(128, dtype=np.float32).astype(BF16)

    # [128, 32]: 16 real head columns + 16 zero columns so each head matmul
    # initializes a full 32-partition PSUM slot (CoreSim rejects uninit reads).
    # [128, 4*128]: four full-width head stationaries; variant m is zero
    # except cols 32m..32m+32, so four macros' heads accumulate into one
    # PSUM bank at disjoint partition ranges (full-array mode, no tiling).
    wos = np.zeros((128, 512), dtype=BF16)
    for m in range(4):
        for g in range(G):
            for o in range(2):
                wos[H * g : H * g + H, 128 * m + 32 * m + 8 * o + g] = (
                    Wo[o, :].astype(BF16)
                )

    # bias vectors [128, 11]: column l = relu layer l (fc1, then A/B per block)
    bvecs = np.zeros((128, 11), dtype=np.float32)
    bvecs[:, 0] = np.tile(b1, G)
    for l in range(10):
        bvecs[:, 1 + l] = np.tile(bh[l], G)

    return w1s, wab, ids, wos, bvecs


def _prep_x(x, ncols=N):
    """x [B,4] f32 -> per-core [n_macros*32, ncols] bf16, partition p=4g+f."""
    # [core, t, g, j, f] -> [core, t, g, f, j]
    xr = x.reshape(N_CORES, -1, G, ncols, 4).transpose(0, 1, 2, 4, 3)
    xr = np.ascontiguousarray(xr).astype(BF16)
    return xr.reshape(N_CORES, -1, ncols)


def _post_y(yparts, bo, ncols=N):
    """per-core [n_macros*16, ncols] f32 -> y [B, 2] (+bo)."""
    y = np.stack(yparts).reshape(N_CORES, -1, 2, G, ncols)
    y = y.transpose(0, 1, 3, 4, 2).reshape(B, 2)
    return y + bo[None, :].astype(np.float32)


def build_module(
    n_macros=N_MACROS,
    num_devices=N_CORES,
    ncols=N,
    skew=4,
    x_bufs=4,
    h_bufs=12,
    y_bufs=2,
    p_bufs=6,
    hd_bufs=2,
    act_set=ACT_PASSES,
    copy_on_dve=False,
    relu_split=0,
    repeat=1,
    hw_loop=0,
    group_b=0,
    pair_skew=0,
    head_merge=0,
    xlead=4,
):
    """Build + compile the per-core Bass module. Returns the compiled nc."""
    nc = bacc.Bacc(
        "TRN2", target_bir_lowering=False, debug=False, num_devices=num_devices
    )
    x_d = nc.dram_tensor("xprep", (n_macros * 32, ncols), _DT_BF16, kind="ExternalInput").ap()
    y_d = nc.dram_tensor("yprep", (n_macros * 16, ncols), _DT_F32, kind="ExternalOutput").ap()
    w1_d = nc.dram_tensor("w1s", (32, 128), _DT_BF16, kind="ExternalInput").ap()
    wab_d = nc.dram_tensor("wab", (128, 1280), _DT_BF16, kind="ExternalInput").ap()
    id_d = nc.dram_tensor("ids", (128, 128), _DT_BF16, kind="ExternalInput").ap()
    wo_d = nc.dram_tensor("wos", (128, 512), _DT_BF16, kind="ExternalInput").ap()
    b_d = nc.dram_tensor("bvecs", (128, 11), _DT_F32, kind="ExternalInput").ap()

    with TileCtx(nc) as (tc, ctx):
        wpool = ctx.enter_context(tc.tile_pool(name="weights", bufs=1))
        w1t = wpool.tile([32, 128], _DT_BF16, tag="w1t")
        nc.sync.dma_start(w1t[:], w1_d[:, :])
        waball = wpool.tile([128, 1280], _DT_BF16, tag="waball")
        nc.sync.dma_start(waball[:], wab_d[:, :])
        wabt = [waball[:, 128 * l : 128 * l + 128] for l in range(10)]
        idt = wpool.tile([128, 128], _DT_BF16, tag="idt")
        nc.sync.dma_start(idt[:], id_d[:, :])
        wot = wpool.tile([128, 512], _DT_BF16, tag="wot")
        nc.sync.dma_start(wot[:], wo_d[:, :])
        ball = wpool.tile([128, 11], _DT_F32, tag="ball")
        nc.sync.dma_start(ball[:], b_d[:, :])
        btiles = [ball[:, l : l + 1] for l in range(11)]

        assert n_macros % 4 == 0
        xpool = ctx.enter_context(tc.tile_pool(name="xin", bufs=x_bufs))
        hpool = ctx.enter_context(tc.tile_pool(name="h", bufs=h_bufs))
        ypool = ctx.enter_context(tc.tile_pool(name="yout", bufs=y_bufs))
        ppool = ctx.enter_context(tc.tile_pool(name="psum", bufs=p_bufs, space="PSUM"))
        if head_merge:
            hdpool = ctx.enter_context(
                tc.tile_pool(name="hd", bufs=hd_bufs, space="PSUM"))
        hd_state = {}

        def act_relu(idx, h, p):
            nc.scalar.activation(
                h, p, mybir.ActivationFunctionType.Relu, bias=btiles[idx]
            )

        def dve_relu(idx, h, p):
            nc.vector.tensor_scalar(
                h, p, btiles[idx], 0.0,
                op0=mybir.AluOpType.add, op1=mybir.AluOpType.max,
            )

        def relu_pass(idx, h, p):
            if relu_split:
                # ScalarE takes the leading columns, VectorE the rest
                act_relu(idx, h[:, 0:relu_split], p[:, 0:relu_split])
                dve_relu(idx, h[:, relu_split:ncols], p[:, relu_split:ncols])
            elif idx in act_set:
                act_relu(idx, h[:], p[:])
            else:
                dve_relu(idx, h[:], p[:])

        # Software-pipelined emission: per-macro state, wavefront schedule.
        st = [dict() for _ in range(n_macros)]

        ncb = ncols // N  # 512-column blocks per tile (PSUM bank per block)
        CB = [slice(N * c, N * c + N) for c in range(ncb)]

        # fine stages: e=0 x-DMA; e=1+2k MM of layer k (k=0..11, 11=head);
        # e=2+2k relu k (k=0..10); e=25 head evacuation.  skew is in fine
        # units (2 fine stages = 1 old coarse stage).
        def stage_fine(t, e):
            m = st[t]
            if e == 0:
                xt = xpool.tile([32, ncols], _DT_BF16, tag="x")
                nc.sync.dma_start(xt[:], x_d[32 * t : 32 * t + 32, :])
                m["x"] = xt
                return
            if e == 24:
                if head_merge:
                    # one [128, ncols] crossing per 4 macros
                    if t % 4 == 3:
                        y4 = ypool.tile([128, ncols], _DT_F32, tag="y")
                        p4 = hd_state.pop("p")
                        if (t // 4) % 2 == 0:
                            nc.scalar.copy(y4[:], p4[:])
                        else:
                            nc.vector.tensor_copy(y4[:], p4[:])
                        hd_state["y"] = y4
                    return
                # evacuate head psum [16, ncols] -> sbuf, alternating engines
                yt = ypool.tile([16, ncols], _DT_F32, tag="y")
                p = m.pop("p")
                if t % 2 == 0:
                    nc.scalar.copy(yt[:], p[0:16, :])
                else:
                    nc.vector.tensor_copy(yt[:], p[0:16, :])
                m["y"] = yt
                return
            if e == 25:
                if head_merge:
                    if t % 4 == 3:
                        y4 = hd_state.pop("y")
                        for j in range(4):
                            nc.sync.dma_start(
                                y_d[16 * (t - 3 + j) : 16 * (t - 3 + j) + 16, :],
                                y4[32 * j : 32 * j + 16, :],
                            )
                    return
                yt = m.pop("y")
                nc.sync.dma_start(y_d[16 * t : 16 * t + 16, :], yt[0:16, :])
                return
            if e % 2 == 1:  # MM of layer k
                k = (e - 1) // 2
                if k == 0:
                    p = ppool.tile([128, ncols], _DT_F32, tag="p")
                    xt = m.pop("x")
                    for c in CB:
                        nc.tensor.matmul(p[:, c], w1t[:], xt[:, c], start=True, stop=True)
                    m["p"] = p
                    return
                if k == 11:  # head
                    h = m[f"h{10}"]
                    if head_merge:
                        mg = t % 4
                        if mg == 0:
                            p4new = hdpool.tile([128, ncols], _DT_F32, tag="hd")
                            hd_state["p"] = p4new
                        p4 = hd_state["p"]
                        for c in CB:
                            nc.tensor.matmul(
                                p4[:, c],
                                wot[:, 128 * mg : 128 * mg + 128],
                                h[:, c],
                                start=(mg == 0), stop=(mg == 3),
                            )
                        return
                    p = ppool.tile([128, ncols], _DT_F32, tag="p")
                    for c in CB:
                        nc.tensor.matmul(
                            p[0:32, c], wot[:, 0:32], h[:, c], start=True, stop=True
                        )
                    m["p"] = p
                    return
                h = m[f"h{k - 1}"]
                p = ppool.tile([128, ncols], _DT_F32, tag="p")
                if k % 2 == 1:  # A-layer
                    for c in CB:
                        nc.tensor.matmul(
                            p[:, c], wabt[k - 1], h[:, c], start=True, stop=True
                        )
                else:  # B-layer + residual id accumulate
                    res = m[f"h{k - 2}"]
                    if group_b:
                        # group by stationary: all Wb blocks, then all id blocks
                        for c in CB:
                            nc.tensor.matmul(
                                p[:, c], wabt[k - 1], h[:, c], start=True, stop=False
                            )
                        for c in CB:
                            nc.tensor.matmul(
                                p[:, c], idt[:], res[:, c], start=False, stop=True
                            )
                    else:
                        for c in CB:
                            nc.tensor.matmul(
                                p[:, c], wabt[k - 1], h[:, c], start=True, stop=False
                            )
                            nc.tensor.matmul(
                                p[:, c], idt[:], res[:, c], start=False, stop=True
                            )
                m["p"] = p
                return
            # relu k
            k = (e - 2) // 2
            if k > 10:
                return
            h = hpool.tile([128, ncols], _DT_BF16, tag="h")
            relu_pass(k, h, m.pop("p"))
            m[f"h{k}"] = h

        # x-DMA (e=0) is scheduled a few steps early so the load is in flight
        # well before the fc1 matmul needs it.
        if pair_skew:
            # pair-lockstep: macros (2t, 2t+1) march together so same-layer
            # matmuls with identical stationaries are emitted adjacently
            # (halves effective LDWEIGHTS cost via weight reuse).
            events = sorted(
                (((t // 2) * pair_skew + (e if e else -xlead), t, e)
                 for t in range(n_macros) for e in range(26)),
                key=lambda ev: (ev[0], ev[1]),
            )
        else:
            events = sorted(
                ((t * skew + (e if e else -xlead), t, e)
                 for t in range(n_macros) for e in range(26)),
                key=lambda ev: (ev[0], ev[1]),
            )
        def emit_all():
            for rep in range(repeat):
                st[:] = [dict() for _ in range(n_macros)]
                hd_state.clear()
                for _, t, e in events:
                    stage_fine(t, e)

        if hw_loop:
            with tc.For_i(0, hw_loop, 1):
                emit_all()
        else:
            emit_all()

    nc.compile()
    return nc


class TileCtx:
    """TileContext + ExitStack in one `with`."""

    def __init__(self, nc):
        self.nc = nc

    def __enter__(self):
        self._es = ExitStack()
        self._tc = self._es.enter_context(tile.TileContext(self.nc))
        return self._tc, self._es

    def __exit__(self, *exc):
        return self._es.__exit__(*exc)


_CACHED_NC = None


def kernel(x, W1, b1, Wh, bh, Wo, bo):
    global _CACHED_NC
    x = np.asarray(x, dtype=np.float32)
    W1 = np.asarray(W1, dtype=np.float32)
    b1 = np.asarray(b1, dtype=np.float32)
    Wh = np.asarray(Wh, dtype=np.float32)
    bh = np.asarray(bh, dtype=np.float32)
    Wo = np.asarray(Wo, dtype=np.float32)
    bo = np.asarray(bo, dtype=np.float32)

    w1s, wab, ids, wos, bvecs = _prep_weights(W1, b1, Wh, bh, Wo, bo)
    xprep = _prep_x(x, ncols=BEST["ncols"])

    if _CACHED_NC is None:
        _CACHED_NC = build_module(num_devices=N_CORES, **BEST)
    nc = _CACHED_NC

    in_maps = [
        {
            "xprep": np.ascontiguousarray(xprep[c]),
            "w1s": w1s,
            "wab": wab,
            "ids": ids,
            "wos": wos,
            "bvecs": bvecs,
        }
        for c in range(N_CORES)
    ]
    res = run_bass_kernel_spmd(nc, in_maps, core_ids=list(range(N_CORES)))
    yparts = [res.results[c]["yprep"] for c in range(N_CORES)]
    return _post_y(yparts, bo, ncols=BEST["ncols"])



# revision 23
# speedup vs baseline: 1.0724x; 1.0724x over previous
"""Trainium2 Bass kernel for the 12-layer residual MLP (nn_Net_40321152975542).

Network (per row of x [B=2097152, 4]):
    h = relu(x @ W1.T + b1)                       # fc1: 4 -> 16
    res = h
    5x: h = relu(h @ Wa.T + ba)                   # A-layer 16 -> 16
        h = relu(h @ Wb.T + bb + res); res = h    # B-layer 16 -> 16 (+residual)
    y = h @ Wo.T + bo                             # head: 16 -> 2

Mapping: pure data-parallel across 8 NeuronCores (batch split).  On each core,
8 batch groups x 16 features are packed across the 128 SBUF partitions and 512
batch elements along the free dim (one "macro tile" = 4096 rows).  Every layer
is one 128x128 block-diagonal bf16 matmul; residual adds are identity matmuls
accumulated into the same PSUM bank; relu+bias runs as a single PSUM->SBUF pass
alternating between ScalarE (activation) and VectorE (tensor_scalar add+max).
The head writes PSUM->DRAM directly; bo is added on the host.

Host-side prep is layout only (transpose/cast/shard) so all device DMAs are
contiguous per partition.
"""

import os
import sys

sys.path.insert(0, "/opt/trn_rl_repo")

import numpy as np
import ml_dtypes
from contextlib import ExitStack

from concourse import bass, bacc, tile, mybir
from concourse.bass_utils import run_bass_kernel_spmd

BF16 = ml_dtypes.bfloat16

B = 2097152
N_CORES = 8
R = B // N_CORES          # rows per core
N = 512                   # free-dim columns per macro tile
G = 8                     # batch groups packed along partitions
H = 16                    # hidden width
MACRO = G * N             # rows per macro tile (4096)
N_MACROS = R // MACRO     # 64

# which relu passes run on ScalarE (rest on VectorE); 11 passes total
ACT_PASSES = frozenset({0, 2, 4, 6, 8, 10})

# best known full-size configuration (see dev_sweep / TimelineSim + HW slope)
BEST = dict(
    ncols=512,
    n_macros=R // (G * 512),
    skew=2,
    p_bufs=6,
    hd_bufs=2,
    h_bufs=14,
    x_bufs=8,
    y_bufs=3,
    head_merge=1,
    xlead=4,
)

_DT_BF16 = mybir.dt.bfloat16
_DT_F32 = mybir.dt.float32


def _prep_weights(W1, b1, Wh, bh, Wo, bo):
    """Build block-diagonal bf16 stationaries + replicated fp32 bias vectors."""
    w1s = np.zeros((32, 128), dtype=BF16)
    for g in range(G):
        w1s[4 * g : 4 * g + 4, H * g : H * g + H] = W1.T.astype(BF16)

    # K-major concat [128, 10*128]: one DMA, layer l stationary = [:, 128l:128l+128]
    wab = np.zeros((128, 10 * 128), dtype=BF16)
    for l in range(10):
        for g in range(G):
            wab[H * g : H * g + H, 128 * l + H * g : 128 * l + H * g + H] = (
                Wh[l].T.astype(BF16)
            )

    ids = np.eye(128, dtype=np.float32).astype(BF16)

    # [128, 32]: 16 real head columns + 16 zero columns so each head matmul
    # initializes a full 32-partition PSUM slot (CoreSim rejects uninit reads).
    # [128, 4*128]: four full-width head stationaries; variant m is zero
    # except cols 32m..32m+32, so four macros' heads accumulate into one
    # PSUM bank at disjoint partition ranges (full-array mode, no tiling).
    wos = np.zeros((128, 512), dtype=BF16)
    for m in range(4):
        for g in range(G):
            for o in range(2):
                wos[H * g : H * g + H, 128 * m + 32 * m + 8 * o + g] = (
                    Wo[o, :].astype(BF16)
                )

    # bias vectors [128, 11]: column l = relu layer l (fc1, then A/B per block)
    bvecs = np.zeros((128, 11), dtype=np.float32)
    bvecs[:, 0] = np.tile(b1, G)
    for l in range(10):
        bvecs[:, 1 + l] = np.tile(bh[l], G)

    return w1s, wab, ids, wos, bvecs


def _prep_x(x, ncols=N):
    """x [B,4] f32 -> per-core [n_macros*32, ncols] bf16, partition p=4g+f."""
    # [core, t, g, j, f] -> [core, t, g, f, j]
    xr = x.reshape(N_CORES, -1, G, ncols, 4).transpose(0, 1, 2, 4, 3)
    xr = np.ascontiguousarray(xr).astype(BF16)
    return xr.reshape(N_CORES, -1, ncols)


def _post_y(yparts, bo, ncols=N):
    """per-core [n_macros*16, ncols] f32 -> y [B, 2] (+bo)."""
    y = np.stack(yparts).reshape(N_CORES, -1, 2, G, ncols)
    y = y.transpose(0, 1, 3, 4, 2).reshape(B, 2)
    return y + bo[None, :].astype(np.float32)


def build_module(
    n_macros=N_MACROS,
    num_devices=N_CORES,
    ncols=N,
    skew=4,
    x_bufs=4,
    h_bufs=12,
    y_bufs=2,
    p_bufs=6,
    hd_bufs=2,
    act_set=ACT_PASSES,
    copy_on_dve=False,
    relu_split=0,
    repeat=1,
    hw_loop=0,
    group_b=0,
    pair_skew=0,
    head_merge=0,
    xlead=4,
):
    """Build + compile the per-core Bass module. Returns the compiled nc."""
    nc = bacc.Bacc(
        "TRN2", target_bir_lowering=False, debug=False, num_devices=num_devices
    )
    x_d = nc.dram_tensor("xprep", (n_macros * 32, ncols), _DT_BF16, kind="ExternalInput").ap()
    y_d = nc.dram_tensor("yprep", (n_macros * 16, ncols), _DT_F32, kind="ExternalOutput").ap()
    w1_d = nc.dram_tensor("w1s", (32, 128), _DT_BF16, kind="ExternalInput").ap()
    wab_d = nc.dram_tensor("wab", (128, 1280), _DT_BF16, kind="ExternalInput").ap()
    id_d = nc.dram_tensor("ids", (128, 128), _DT_BF16, kind="ExternalInput").ap()
    wo_d = nc.dram_tensor("wos", (128, 512), _DT_BF16, kind="ExternalInput").ap()
    b_d = nc.dram_tensor("bvecs", (128, 11), _DT_F32, kind="ExternalInput").ap()

    with TileCtx(nc) as (tc, ctx):
        wpool = ctx.enter_context(tc.tile_pool(name="weights", bufs=1))
        w1t = wpool.tile([32, 128], _DT_BF16, tag="w1t")
        nc.sync.dma_start(w1t[:], w1_d[:, :])
        waball = wpool.tile([128, 1280], _DT_BF16, tag="waball")
        nc.sync.dma_start(waball[:], wab_d[:, :])
        wabt = [waball[:, 128 * l : 128 * l + 128] for l in range(10)]
        idt = wpool.tile([128, 128], _DT_BF16, tag="idt")
        nc.sync.dma_start(idt[:], id_d[:, :])
        wot = wpool.tile([128, 512], _DT_BF16, tag="wot")
        nc.sync.dma_start(wot[:], wo_d[:, :])
        ball = wpool.tile([128, 11], _DT_F32, tag="ball")
        nc.sync.dma_start(ball[:], b_d[:, :])
        btiles = [ball[:, l : l + 1] for l in range(11)]

        assert n_macros % 4 == 0
        xpool = ctx.enter_context(tc.tile_pool(name="xin", bufs=x_bufs))
        hpool = ctx.enter_context(tc.tile_pool(name="h", bufs=h_bufs))
        ypool = ctx.enter_context(tc.tile_pool(name="yout", bufs=y_bufs))
        ppool = ctx.enter_context(tc.tile_pool(name="psum", bufs=p_bufs, space="PSUM"))
        if head_merge:
            hdpool = ctx.enter_context(
                tc.tile_pool(name="hd", bufs=hd_bufs, space="PSUM"))
        hd_state = {}

        def act_relu(idx, h, p):
            nc.scalar.activation(
                h, p, mybir.ActivationFunctionType.Relu, bias=btiles[idx]
            )

        def dve_relu(idx, h, p):
            nc.vector.tensor_scalar(
                h, p, btiles[idx], 0.0,
                op0=mybir.AluOpType.add, op1=mybir.AluOpType.max,
            )

        def relu_pass(idx, h, p):
            if relu_split:
                # ScalarE takes the leading columns, VectorE the rest
                act_relu(idx, h[:, 0:relu_split], p[:, 0:relu_split])
                dve_relu(idx, h[:, relu_split:ncols], p[:, relu_split:ncols])
            elif idx in act_set:
                act_relu(idx, h[:], p[:])
            else:
                dve_relu(idx, h[:], p[:])

        # Software-pipelined emission: per-macro state, wavefront schedule.
        st = [dict() for _ in range(n_macros)]

        ncb = ncols // N  # 512-column blocks per tile (PSUM bank per block)
        CB = [slice(N * c, N * c + N) for c in range(ncb)]

        # fine stages: e=0 x-DMA; e=1+2k MM of layer k (k=0..11, 11=head);
        # e=2+2k relu k (k=0..10); e=25 head evacuation.  skew is in fine
        # units (2 fine stages = 1 old coarse stage).
        def stage_fine(t, e):
            m = st[t]
            if e == 0:
                xt = xpool.tile([32, ncols], _DT_BF16, tag="x")
                nc.sync.dma_start(xt[:], x_d[32 * t : 32 * t + 32, :])
                m["x"] = xt
                return
            if e == 24:
                if head_merge:
                    # one [128, ncols] crossing per 4 macros
                    if t % 4 == 3:
                        y4 = ypool.tile([128, ncols], _DT_F32, tag="y")
                        p4 = hd_state.pop("p")
                        if (t // 4) % 2 == 0:
                            nc.scalar.copy(y4[:], p4[:])
                        else:
                            nc.vector.tensor_copy(y4[:], p4[:])
                        hd_state["y"] = y4
                    return
                # evacuate head psum [16, ncols] -> sbuf, alternating engines
                yt = ypool.tile([16, ncols], _DT_F32, tag="y")
                p = m.pop("p")
                if t % 2 == 0:
                    nc.scalar.copy(yt[:], p[0:16, :])
                else:
                    nc.vector.tensor_copy(yt[:], p[0:16, :])
                m["y"] = yt
                return
            if e == 25:
                if head_merge:
                    if t % 4 == 3:
                        y4 = hd_state.pop("y")
                        for j in range(4):
                            nc.sync.dma_start(
                                y_d[16 * (t - 3 + j) : 16 * (t - 3 + j) + 16, :],
                                y4[32 * j : 32 * j + 16, :],
                            )
                    return
                yt = m.pop("y")
                nc.sync.dma_start(y_d[16 * t : 16 * t + 16, :], yt[0:16, :])
                return
            if e % 2 == 1:  # MM of layer k
                k = (e - 1) // 2
                if k == 0:
                    p = ppool.tile([128, ncols], _DT_F32, tag="p")
                    xt = m.pop("x")
                    for c in CB:
                        nc.tensor.matmul(p[:, c], w1t[:], xt[:, c], start=True, stop=True)
                    m["p"] = p
                    return
                if k == 11:  # head
                    h = m[f"h{10}"]
                    if head_merge:
                        mg = t % 4
                        if mg == 0:
                            p4new = hdpool.tile([128, ncols], _DT_F32, tag="hd")
                            hd_state["p"] = p4new
                        p4 = hd_state["p"]
                        for c in CB:
                            nc.tensor.matmul(
                                p4[:, c],
                                wot[:, 128 * mg : 128 * mg + 128],
                                h[:, c],
                                start=(mg == 0), stop=(mg == 3),
                            )
                        return
                    p = ppool.tile([128, ncols], _DT_F32, tag="p")
                    for c in CB:
                        nc.tensor.matmul(
                            p[0:32, c], wot[:, 0:32], h[:, c], start=True, stop=True
                        )
                    m["p"] = p
                    return
                h = m[f"h{k - 1}"]
                p = ppool.tile([128, ncols], _DT_F32, tag="p")
                if k % 2 == 1:  # A-layer
                    for c in CB:
                        nc.tensor.matmul(
                            p[:, c], wabt[k - 1], h[:, c], start=True, stop=True
                        )
                else:  # B-layer + residual id accumulate
                    res = m[f"h{k - 2}"]
                    if group_b:
                        # group by stationary: all Wb blocks, then all id blocks
                        for c in CB:
                            nc.tensor.matmul(
                                p[:, c], wabt[k - 1], h[:, c], start=True, stop=False
                            )
                        for c in CB:
                            nc.tensor.matmul(
                                p[:, c], idt[:], res[:, c], start=False, stop=True
                            )
                    else:
                        for c in CB:
                            nc.tensor.matmul(
                                p[:, c], wabt[k - 1], h[:, c], start=True, stop=False
                            )
                            nc.tensor.matmul(
                                p[:, c], idt[:], res[:, c], start=False, stop=True
                            )
                m["p"] = p
                return
            # relu k
            k = (e - 2) // 2
            if k > 10:
                return
            h = hpool.tile([128, ncols], _DT_BF16, tag="h")
            relu_pass(k, h, m.pop("p"))
            m[f"h{k}"] = h

        # x-DMA (e=0) is scheduled a few steps early so the load is in flight
        # well before the fc1 matmul needs it.
        if pair_skew:
            # pair-lockstep: macros (2t, 2t+1) march together so same-layer
            # matmuls with identical stationaries are emitted adjacently
            # (halves effective LDWEIGHTS cost via weight reuse).
            events = sorted(
                (((t // 2) * pair_skew + (e if e else -xlead), t, e)
                 for t in range(n_macros) for e in range(26)),
                key=lambda ev: (ev[0], ev[1]),
            )
        else:
            events = sorted(
                ((t * skew + (e if e else -xlead), t, e)
                 for t in range(n_macros) for e in range(26)),
                key=lambda ev: (ev[0], ev[1]),
            )
        def emit_all():
            for rep in range(repeat):
                st[:] = [dict() for _ in range(n_macros)]
                hd_state.clear()
                for _, t, e in events:
                    stage_fine(t, e)

        if hw_loop:
            with tc.For_i(0, hw_loop, 1):
                emit_all()
        else:
            emit_all()

    nc.compile()
    return nc


class TileCtx:
    """TileContext + ExitStack in one `with`."""

    def __init__(self, nc):
        self.nc = nc

    def __enter__(self):
        self._es = ExitStack()
        self._tc = self._es.enter_context(tile.TileContext(self.nc))
        return self._tc, self._es

    def __exit__(self, *exc):
        return self._es.__exit__(*exc)


_CACHED_NC = None


def kernel(x, W1, b1, Wh, bh, Wo, bo):
    global _CACHED_NC
    x = np.asarray(x, dtype=np.float32)
    W1 = np.asarray(W1, dtype=np.float32)
    b1 = np.asarray(b1, dtype=np.float32)
    Wh = np.asarray(Wh, dtype=np.float32)
    bh = np.asarray(bh, dtype=np.float32)
    Wo = np.asarray(Wo, dtype=np.float32)
    bo = np.asarray(bo, dtype=np.float32)

    w1s, wab, ids, wos, bvecs = _prep_weights(W1, b1, Wh, bh, Wo, bo)
    xprep = _prep_x(x, ncols=BEST["ncols"])

    if _CACHED_NC is None:
        _CACHED_NC = build_module(num_devices=N_CORES, **BEST)
    nc = _CACHED_NC

    in_maps = [
        {
            "xprep": np.ascontiguousarray(xprep[c]),
            "w1s": w1s,
            "wab": wab,
            "ids": ids,
            "wos": wos,
            "bvecs": bvecs,
        }
        for c in range(N_CORES)
    ]
    res = run_bass_kernel_spmd(nc, in_maps, core_ids=list(range(N_CORES)))
    yparts = [res.results[c]["yprep"] for c in range(N_CORES)]
    return _post_y(yparts, bo, ncols=BEST["ncols"])

